# revision 26
# baseline (speedup 1.0000x reference)
"""MiniBert (embed + LayerNorm + single-head attention) on 8 TRN2 NeuronCores.

Strategy: data-parallel over batch (4 sequences per core), embedding table /
pos table / projection weights replicated to every core.

Algebraic fusion: softmax(q k^T) with q = x Wq/sqrt(D), k = x Wk is
softmax(x M x^T) with M = Wq Wk^T/sqrt(D) precomputed on host (gamma folded
in, double-centered — valid because LayerNorm rows are zero-mean) — the K
projection disappears from the device entirely.

Software pipeline (steady state): while sequence b runs attention on the PE
(S = A^T.T @ x^T -> row-max -> exp -> P^T via DMA transpose -> O = P@V), the
next sequence's embedding gathers, LayerNorm (Pool/DVE/ACT), x^T transposes,
V tiles and A^T chunks are woven into the same engine streams so every
engine's in-order queue stays busy and the PE never waits on the softmax
round-trip.
"""
import math
import numpy as np

from concourse import bass, mybir
import concourse.tile as tile
from concourse.bass_utils import run_bass_kernel_spmd
from concourse.masks import make_identity

P = 128
D = 512
VOC = 32000
N_CORES = 8

F32 = mybir.dt.float32
F32R = mybir.dt.float32r
F16 = mybir.dt.float16
I32 = mybir.dt.int32

AF = mybir.ActivationFunctionType
ALU = mybir.AluOpType
AX = mybir.AxisListType


def fix_fat_waits(nc, max_waits=1):
    """Walrus rejects instructions carrying more than ~1 semaphore wait. Tile
    occasionally emits joins (notably the kernel-tail drain) with one wait per
    producing processor. Split the extras into a chain of single-wait NoOps on
    the same engine, inserted immediately before the original instruction."""
    n_new = 0
    for bb in nc.main_func.blocks:
        insts = bb.instructions
        i = 0
        while i < len(insts):
            ins = insts[i]
            si = ins.sync_info
            if si and si.on_wait and len(si.on_wait) > max_waits:
                waits = list(si.on_wait)
                keep = waits[-max_waits:]
                extra = waits[:-max_waits]
                ins.sync_info = mybir.SyncInfo(
                    on_wait=keep, on_update=list(si.on_update or []))
                for j, w in enumerate(extra):
                    nop = mybir.InstNoOp(name=f"W-split-{n_new}", ins=[], outs=[])
                    n_new += 1
                    nop.engine = ins.engine
                    nop.sync_info = mybir.SyncInfo(on_wait=[w], on_update=[])
                    insts.insert(i + j, nop)
                i += len(extra)
            i += 1
    return n_new


def build(b_per_core: int, s_len: int, voc: int, apply_beta: bool,
          debug: bool = False, stages: int = 99,
          bufs_ebuf: int = 2, bufs_sm: int = 3, bufs_psm: int = 2,
          bufs_pss: int = 3):
    """Build the per-core SPMD program. All cores run this same module."""
    nt = s_len // P          # token tiles per sequence (8)
    dk = D // P              # feature tiles (4)
    nchunk = 2               # moving-dim chunks for N=s_len matmuls (512 each)
    ckw = s_len // nchunk    # 512

    nc = bass.Bass()

    ids_d = nc.dram_tensor("ids", [P, b_per_core * nt], I32, kind="ExternalInput")
    wemb_d = nc.dram_tensor("wemb", [voc, D], F32, kind="ExternalInput")
    pos_d = nc.dram_tensor("pos", [s_len, D], F32, kind="ExternalInput")
    wm_d = nc.dram_tensor("wm", [D, D], F32, kind="ExternalInput")
    wv_d = nc.dram_tensor("wv", [D, D], F32, kind="ExternalInput")
    if apply_beta:
        beta_d = nc.dram_tensor("beta_b", [P, D], F32, kind="ExternalInput")
        abias_d = nc.dram_tensor("abias", [P, dk], F32, kind="ExternalInput")
    out_d = nc.dram_tensor("out", [b_per_core * s_len, D], F32, kind="ExternalOutput")
    scr_d = nc.dram_tensor("scr", [P, 2], F32, kind="ExternalOutput")

    stages_eff = 99 if stages < 0 else stages

    with tile.TileContext(nc) as tc:
        with (
            tc.tile_pool(name="pers", bufs=1) as pers,
            tc.tile_pool(name="ebuf", bufs=bufs_ebuf) as ebuf,
            tc.tile_pool(name="proj", bufs=2) as proj,
            tc.tile_pool(name="sm", bufs=bufs_sm) as sm,
            tc.tile_pool(name="ps_misc", bufs=bufs_psm, space="PSUM") as ps_misc,
            tc.tile_pool(name="ps_s", bufs=bufs_pss, space="PSUM") as ps_s,
        ):
            # ------------- preamble, ordered for the serial DMA pipe -------
            # emission order decides who wins the shared DMA data mover, so:
            # ids -> pos(first half) -> batch-0 gathers -> pos(second half)
            # -> weights. Small gpsimd memsets go first (Pool engine is the
            # gather issuer).
            ident_f = pers.tile([P, P], F32, tag="ident_f")
            make_identity(nc, ident_f[:])
            ident = pers.tile([P, P], F32R, tag="ident")
            nc.vector.tensor_copy(out=ident[:], in_=ident_f[:])
            epsb = pers.tile([P, 1], F32, tag="epsb")
            nc.gpsimd.memset(epsb[:], 1e-5)

            ids_t = pers.tile([P, b_per_core * nt], I32, tag="ids")
            nc.sync.dma_start(out=ids_t[:], in_=ids_d[:, :])

            pos_t = pers.tile([P, nt, D], F32, tag="pos")
            pos_r = pos_d.rearrange("(a p) d -> p a d", p=P)
            nc.sync.dma_start(out=pos_t[:], in_=pos_r[:, :, :])


            # ---- per-batch tile sets ----
            def alloc_tiles(b):
                t = {}
                t["e_all"] = ebuf.tile([P, nt, D], F32, tag="e_all",
                                       name=f"e_all_{b}")
                t["s6"] = ebuf.tile([P, nt, 6], F32, tag="s6", name=f"s6_{b}")
                t["mv"] = ebuf.tile([P, nt, 2], F32, tag="mv", name=f"mv_{b}")
                t["lnv"] = ebuf.tile([P, nt], F32, tag="lnv", name=f"lnv_{b}")
                t["rs"] = ebuf.tile([P, nt], F32, tag="rs", name=f"rs_{b}")
                t["nmurs"] = ebuf.tile([P, nt], F32, tag="nmurs",
                                       name=f"nmurs_{b}")
                t["x_all"] = ebuf.tile([P, nt, D], F32R, tag="x_all",
                                       name=f"x_all_{b}")
                t["xt"] = ebuf.tile([P, dk, s_len], F32R, tag="xt",
                                    name=f"xt_{b}")
                t["at"] = proj.tile([P, dk, s_len], F32R, tag="at",
                                    name=f"at_{b}")
                t["v16"] = proj.tile([P, nt, D], F16, tag="v16",
                                     name=f"v16_{b}")
                t["nm"] = ebuf.tile([P, nt], F32, tag="nm", name=f"nm_{b}")
                t["ls"] = ebuf.tile([P, nt], F32, tag="ls", name=f"ls_{b}")
                t["rr"] = ebuf.tile([P, nt], F32, tag="rr", name=f"rr_{b}")
                t["s_ps"] = [None] * nt
                t["pt16"] = [None] * nt
                return t

            def emit_gathers(b, t):
                if stages_eff < 1:
                    return
                for j in range(nt):
                    nc.gpsimd.indirect_dma_start(
                        out=t["e_all"][:, j, :],
                        out_offset=None,
                        in_=wemb_d[:],
                        in_offset=bass.IndirectOffsetOnAxis(
                            ap=ids_t[:, b * nt + j: b * nt + j + 1], axis=0),
                    )

            def emit_ln(b, t, j, posadd_on_pool):
                """LayerNorm chain for token tile j (no PE work). DVE does
                only DVE-local work (stats + negated mean); the rs-dependent
                products run on ACT so neither engine's in-order queue ever
                blocks on the other's round-trip."""
                if stages_eff < 2:
                    return
                eng = nc.gpsimd if posadd_on_pool else nc.vector
                eng.tensor_tensor(
                    out=t["e_all"][:, j, :], in0=t["e_all"][:, j, :],
                    in1=pos_t[:, j, :], op=ALU.add)
                nc.vector.bn_stats(out=t["s6"][:, j, :], in_=t["e_all"][:, j, :])
                nc.vector.bn_aggr(out=t["mv"][:, j, :], in_=t["s6"][:, j, :])
                # nmu = -mean (DVE-local)
                nc.vector.tensor_scalar_mul(
                    out=t["nmurs"][:, j:j + 1], in0=t["mv"][:, j, 0:1],
                    scalar1=-1.0)
                # rs = exp(-0.5*ln(var + eps)) == rsqrt(var + eps)
                nc.scalar.activation(
                    out=t["lnv"][:, j:j + 1], in_=t["mv"][:, j, 1:2],
                    func=AF.Ln, bias=epsb[:, 0:1], scale=1.0)
                nc.scalar.activation(
                    out=t["rs"][:, j:j + 1], in_=t["lnv"][:, j:j + 1],
                    func=AF.Exp, bias=0.0, scale=-0.5)
                # nmurs = (-mean) * rs, on ACT (scale is a per-partition AP)
                nc.scalar.activation(
                    out=t["nmurs"][:, j:j + 1], in_=t["nmurs"][:, j:j + 1],
                    func=AF.Identity, bias=0.0, scale=t["rs"][:, j:j + 1])
                if apply_beta:
                    xtmp = ebuf.tile([P, D], F32, tag="xtmp",
                                     name=f"xtmp_{b}_{j}")
                    nc.scalar.activation(
                        out=xtmp[:], in_=t["e_all"][:, j, :], func=AF.Identity,
                        bias=t["nmurs"][:, j:j + 1], scale=t["rs"][:, j:j + 1])
                    nc.vector.tensor_tensor(
                        out=t["x_all"][:, j, :], in0=xtmp[:], in1=beta_t[:],
                        op=ALU.add)
                else:
                    nc.scalar.activation(
                        out=t["x_all"][:, j, :], in_=t["e_all"][:, j, :],
                        func=AF.Identity,
                        bias=t["nmurs"][:, j:j + 1], scale=t["rs"][:, j:j + 1])

            def emit_t(b, t, j):
                """x^T transpose for token tile j (PE + DVE copy)."""
                if stages_eff < 3:
                    return
                pst = ps_misc.tile([P, dk, P], F32R, tag="ps_misc",
                                   name=f"pst_{b}_{j}")
                for c in range(dk):
                    nc.tensor.transpose(
                        out=pst[:, c, :],
                        in_=t["x_all"][:, j, c * P:(c + 1) * P],
                        identity=ident[:])
                nc.vector.tensor_copy(
                    out=t["xt"][:, :, j * P:(j + 1) * P], in_=pst[:])

            def emit_v(b, t, j):
                """V tile j = x^T[:, j-window]^T @ Wv' (PE work). Emitted at
                least one unit after emit_t(j) so the xt copy has slack."""
                if stages_eff < 4:
                    return
                psv = ps_misc.tile([P, D], F32, tag="ps_misc",
                                   name=f"psv_{b}_{j}")
                for di in range(dk):
                    nc.tensor.matmul(
                        out=psv[:],
                        lhsT=t["xt"][:, di, j * P:(j + 1) * P],
                        rhs=wv_r[:, di, :],
                        start=(di == 0), stop=(di == dk - 1))
                nc.scalar.copy(out=t["v16"][:, j, :], in_=psv[:])

            def emit_a_chunk(b, t, ch):
                """A^T[:, :, ch-window] = sum_di M'[di]^T @ x^T[di]."""
                if stages_eff < 4:
                    return
                for dj in range(dk):
                    ps = ps_misc.tile([P, ckw], F32, tag="ps_misc",
                                      name=f"psa_{b}_{ch}_{dj}")
                    for di in range(dk):
                        nc.tensor.matmul(
                            out=ps[:],
                            lhsT=wm_r[:, di, dj * P:(dj + 1) * P],
                            rhs=t["xt"][:, di, ch * ckw:(ch + 1) * ckw],
                            start=(di == 0), stop=(di == dk - 1))
                    dst = t["at"][:, dj, ch * ckw:(ch + 1) * ckw]
                    if apply_beta:
                        nc.scalar.activation(
                            out=dst, in_=ps[:], func=AF.Identity,
                            bias=abias_t[:, dj:dj + 1], scale=1.0)
                    else:
                        nc.scalar.copy(out=dst, in_=ps[:])

            # ---- attention stages ----
            def attn_a(b, t, j):
                if stages_eff < 5:
                    return
                s_ps = ps_s.tile([P, nchunk, 512], F32, tag="s_ps",
                                 name=f"sps_{b}_{j}")
                t["s_ps"][j] = s_ps
                for h in range(dk):
                    for ch in range(nchunk):
                        nc.tensor.matmul(
                            out=s_ps[:, ch, :ckw],
                            lhsT=t["at"][:, h, j * P:(j + 1) * P],
                            rhs=t["xt"][:, h, ch * ckw:(ch + 1) * ckw],
                            start=(h == 0), stop=(h == dk - 1))

            def attn_b(b, t, j):
                if stages_eff < 5:
                    return
                s_ps = t["s_ps"][j]
                nc.vector.tensor_reduce(
                    out=t["nm"][:, j:j + 1], in_=s_ps[:, :, :ckw], axis=AX.XY,
                    op=ALU.max, negate=True)
                p16 = sm.tile([P, s_len], F16, tag="p16", name=f"p16_{b}_{j}")
                nc.scalar.activation(
                    out=p16[:].rearrange("p (a d) -> p a d", a=nchunk),
                    in_=s_ps[:, :, :ckw], func=AF.Exp,
                    bias=t["nm"][:, j:j + 1], scale=1.0,
                    accum_out=t["ls"][:, j:j + 1])
                nc.vector.reciprocal(
                    out=t["rr"][:, j:j + 1], in_=t["ls"][:, j:j + 1])
                pt16 = sm.tile([P, nt, P], F16, tag="pt16",
                               name=f"pt16_{b}_{j}")
                t["pt16"][j] = pt16
                nc.sync.dma_start_transpose(pt16[:], p16[:])

            def attn_c(b, t, j):
                if stages_eff < 6:
                    return
                pt16 = t["pt16"][j]
                o_ps = ps_misc.tile([P, D], F32, tag="ps_misc",
                                    name=f"ops_{b}_{j}")
                for k in range(nt):
                    nc.tensor.matmul(
                        out=o_ps[:],
                        lhsT=pt16[:, k, :],
                        rhs=t["v16"][:, k, :],
                        start=(k == 0), stop=(k == nt - 1))
                o_sb = sm.tile([P, D], F32, tag="o_sb", name=f"osb_{b}_{j}")
                nc.vector.tensor_scalar_mul(
                    out=o_sb[:], in0=o_ps[:], scalar1=t["rr"][:, j:j + 1])
                row = (b * nt + j) * P
                nc.sync.dma_start(out=out_d[row:row + P, :], in_=o_sb[:])

            def attn_interleaved(b, t, t_next):
                """Attention for batch b with the next batch's prep units
                woven in. Per q-tile step: S(j), softmax(j), PV(j-2), plus
                one LN unit and (late steps) a transpose or V unit of b+1."""
                if stages_eff < 5:
                    # no attention: still need next batch's prep emitted
                    if t_next is not None:
                        for j in range(nt):
                            emit_ln(b + 1, t_next, j, posadd_on_pool=True)
                            emit_t(b + 1, t_next, j)
                            if j > 0:
                                emit_v(b + 1, t_next, j - 1)
                            for ch in range(nchunk):
                                if j == (ch + 1) * (nt // nchunk) - 1:
                                    emit_a_chunk(b + 1, t_next, ch)
                        emit_v(b + 1, t_next, nt - 1)
                    return
                if t_next is None or nt != 8:
                    # no prep fill (or non-tuned shape): stagger PV behind S
                    # so the softmax+transpose round-trip stays covered by
                    # queued S work.
                    stag = min(3, nt - 1)
                    for j in range(nt):
                        attn_a(b, t, j)
                        attn_b(b, t, j)
                        if j >= stag:
                            attn_c(b, t, j - stag)
                    for j in range(nt - stag, nt):
                        attn_c(b, t, j)
                    if t_next is not None:
                        for j in range(nt):
                            emit_ln(b + 1, t_next, j, posadd_on_pool=True)
                            emit_t(b + 1, t_next, j)
                            if j > 0:
                                emit_v(b + 1, t_next, j - 1)
                            for ch in range(nchunk):
                                if j == (ch + 1) * (nt // nchunk) - 1:
                                    emit_a_chunk(b + 1, t_next, ch)
                        emit_v(b + 1, t_next, nt - 1)
                    return
                attn_a(b, t, 0)
                attn_b(b, t, 0)
                attn_a(b, t, 1)
                attn_b(b, t, 1)
                for j in range(2, nt):
                    attn_a(b, t, j)
                    attn_b(b, t, j)
                    attn_c(b, t, j - 2)
                    emit_ln(b + 1, t_next, j - 2, posadd_on_pool=True)
                    if j >= 4:
                        emit_t(b + 1, t_next, j - 4)
                    if j >= 5:
                        emit_v(b + 1, t_next, j - 5)
                # tail: remaining prep of b+1 covers the last softmax
                # round-trips; A^T chunks right after their x^T windows.
                emit_ln(b + 1, t_next, 6, posadd_on_pool=True)
                emit_t(b + 1, t_next, 4)
                emit_v(b + 1, t_next, 3)
                attn_c(b, t, nt - 2)
                emit_ln(b + 1, t_next, 7, posadd_on_pool=True)
                emit_t(b + 1, t_next, 5)
                emit_v(b + 1, t_next, 4)
                emit_a_chunk(b + 1, t_next, 0)
                attn_c(b, t, nt - 1)
                emit_t(b + 1, t_next, 6)
                emit_v(b + 1, t_next, 5)
                emit_t(b + 1, t_next, 7)
                emit_v(b + 1, t_next, 6)
                emit_a_chunk(b + 1, t_next, 1)
                emit_v(b + 1, t_next, 7)

            # ------------- batch 0 prep (cold start) -------------
            wv_r = wm_r = beta_t = abias_t = None

            def emit_beta():
                nonlocal beta_t, abias_t
                if apply_beta:
                    beta_t = pers.tile([P, D], F32, tag="betab", name="betab")
                    nc.sync.dma_start(out=beta_t[:], in_=beta_d[:, :])
                    abias_t = pers.tile([P, dk], F32, tag="abias",
                                        name="abias")
                    nc.sync.dma_start(out=abias_t[:], in_=abias_d[:, :])

            def emit_cvts(gate=None):
                # f32 -> f32r needs a rounding compute op (BIR verifier
                # rejects DMA-fed f32r). Stage through one shared buffer;
                # the copies run on the Pool engine after its gather
                # descriptor generation, off every critical chain. When
                # `gate` (the cold-start e_all tile) is given, tiny marker
                # DMAs make the weight loads WAIT on early gathers, so the
                # serial DMA data mover serves the LayerNorm-critical gather
                # tiles before the bulky weights.
                nonlocal wv_r, wm_r
                w_st = pers.tile([P, dk, D], F32, tag="w_st", name="w_st")
                wv_r = pers.tile([P, dk, D], F32R, tag="wvr", name="wvr")
                if gate is not None:
                    nc.sync.dma_start(out=scr_d[:, 0:1], in_=gate[:, 1, 0:1])
                nc.sync.dma_start(
                    out=w_st[:], in_=wv_d.rearrange("(a p) n -> p a n", p=P))
                nc.gpsimd.tensor_copy(out=wv_r[:], in_=w_st[:])
                wm_r = pers.tile([P, dk, D], F32R, tag="wmr", name="wmr")
                if gate is not None:
                    nc.sync.dma_start(out=scr_d[:, 1:2], in_=gate[:, 3, 0:1])
                nc.sync.dma_start(
                    out=w_st[:], in_=wm_d.rearrange("(a p) n -> p a n", p=P))
                nc.gpsimd.tensor_copy(out=wm_r[:], in_=w_st[:])

            tiles = [None] * max(b_per_core, 1)
            if stages < 0:
                # hardware-loop variant (unused by the test harness's timing
                # path for stages>=0): simple non-pipelined loop
                emit_beta()
                emit_cvts()
                with tc.For_i(0, -stages, 1):
                    t0 = alloc_tiles(0)
                    emit_gathers(0, t0)
                    for j in range(nt):
                        emit_ln(0, t0, j, posadd_on_pool=False)
                        emit_t(0, t0, j)
                        if j > 0:
                            emit_v(0, t0, j - 1)
                        for ch in range(nchunk):
                            if j == (ch + 1) * (nt // nchunk) - 1:
                                emit_a_chunk(0, t0, ch)
                    emit_v(0, t0, nt - 1)
                    attn_interleaved(0, t0, None)
            else:
                emit_beta()
                tiles[0] = alloc_tiles(0)
                emit_gathers(0, tiles[0])
                emit_cvts(gate=tiles[0]["e_all"] if nt >= 4 else None)

                t0 = tiles[0]
                if nt == 8 and stages_eff >= 5:
                    # cold start, ordered to match DMA-pipe arrivals:
                    # first 4 token tiles -> transposes; V+A^T ch0 as the
                    # gated weights land; S/softmax of q-tiles 0..3 (need
                    # only ch0); then tiles 4..7; PV once V is complete.
                    for j in range(nt):
                        emit_ln(0, t0, j, posadd_on_pool=False)
                    for j in range(4):
                        emit_t(0, t0, j)
                    for j in range(4):
                        emit_v(0, t0, j)
                    emit_a_chunk(0, t0, 0)
                    for j in range(4):
                        attn_a(0, t0, j)
                        attn_b(0, t0, j)
                    for j in range(4, nt):
                        emit_t(0, t0, j)
                    for j in range(4, nt):
                        emit_v(0, t0, j)
                    emit_a_chunk(0, t0, 1)
                    attn_c(0, t0, 0)
                    if b_per_core > 1:
                        tiles[1] = alloc_tiles(1)
                        t1 = tiles[1]
                        emit_gathers(1, t1)
                        for j in range(4, nt):
                            attn_a(0, t0, j)
                            attn_b(0, t0, j)
                            attn_c(0, t0, j - 3)
                            emit_ln(1, t1, j - 4, posadd_on_pool=True)
                            if j >= 6:
                                emit_t(1, t1, j - 6)
                            if j == 7:
                                emit_v(1, t1, 0)
                        emit_ln(1, t1, 4, posadd_on_pool=True)
                        emit_t(1, t1, 2)
                        emit_v(1, t1, 1)
                        attn_c(0, t0, 5)
                        emit_ln(1, t1, 5, posadd_on_pool=True)
                        emit_t(1, t1, 3)
                        emit_v(1, t1, 2)
                        attn_c(0, t0, 6)
                        emit_ln(1, t1, 6, posadd_on_pool=True)
                        emit_t(1, t1, 4)
                        emit_v(1, t1, 3)
                        emit_a_chunk(1, t1, 0)
                        attn_c(0, t0, 7)
                        emit_ln(1, t1, 7, posadd_on_pool=True)
                        emit_t(1, t1, 5)
                        emit_v(1, t1, 4)
                        emit_t(1, t1, 6)
                        emit_v(1, t1, 5)
                        emit_t(1, t1, 7)
                        emit_v(1, t1, 6)
                        emit_a_chunk(1, t1, 1)
                        emit_v(1, t1, 7)
                    else:
                        for j in range(4, nt):
                            attn_a(0, t0, j)
                            attn_b(0, t0, j)
                            attn_c(0, t0, j - 3)
                        attn_c(0, t0, 5)
                        attn_c(0, t0, 6)
                        attn_c(0, t0, 7)
                    first_attn = 1
                else:
                    for j in range(nt):
                        emit_ln(0, t0, j, posadd_on_pool=False)
                        emit_t(0, t0, j)
                    for j in range(nt):
                        emit_v(0, t0, j)
                        for ch in range(nchunk):
                            if j == (ch + 1) * (nt // nchunk) - 1:
                                emit_a_chunk(0, t0, ch)
                    first_attn = 0

                for b in range(first_attn, b_per_core):
                    nxt = b + 1 if b + 1 < b_per_core else None
                    if nxt is not None:
                        if tiles[nxt] is None:
                            tiles[nxt] = alloc_tiles(nxt)
                            emit_gathers(nxt, tiles[nxt])
                        attn_interleaved(b, tiles[b], tiles[nxt])
                    else:
                        attn_interleaved(b, tiles[b], None)

    fix_fat_waits(nc)
    return nc


_CACHE = {}


def _get_module(b_per_core, s_len, voc, apply_beta, stages=99):
    key = (b_per_core, s_len, voc, apply_beta, stages)
    if key not in _CACHE:
        _CACHE[key] = build(b_per_core, s_len, voc, apply_beta, stages=stages)
    return _CACHE[key]


def prepare_in_maps(input, word_emb, pos_emb, gamma, beta, Wk, Wq, Wv,
                    b_per_core):
    """Host-side preprocessing -> per-core input maps (one dict per core)."""
    input = np.asarray(input)
    word_emb = np.ascontiguousarray(np.asarray(word_emb, dtype=np.float32))
    pos_emb = np.asarray(pos_emb, dtype=np.float32)
    gamma = np.asarray(gamma, dtype=np.float32)
    beta = np.asarray(beta, dtype=np.float32)
    Wk = np.asarray(Wk, dtype=np.float32)
    Wq = np.asarray(Wq, dtype=np.float32)
    Wv = np.asarray(Wv, dtype=np.float32)

    B, S = input.shape
    nt = S // P
    dk = D // P

    # Fused logit matrix M = (g*Wq)(g*Wk)^T / sqrt(D): softmax(q k^T) ==
    # softmax(x_dev M x_dev^T). Double-center M (rows of x_dev sum to the
    # host-known constant sum(beta/gamma); the LayerNorm part sums to zero)
    # to keep fp32r matmul operands small and well-conditioned.
    g64 = gamma.astype(np.float64)
    b64 = beta.astype(np.float64) / np.where(g64 == 0.0, 1.0, g64)
    wq64 = Wq.astype(np.float64) * g64[:, None]
    wk64 = Wk.astype(np.float64) * g64[:, None]
    m64 = wq64 @ wk64.T / math.sqrt(D)
    cm = m64.mean(0, keepdims=True)
    rm = m64.mean(1, keepdims=True)
    gm = m64.mean()
    wm_s = np.ascontiguousarray((m64 - cm - rm + gm).astype(np.float32))
    wv_s = (Wv.astype(np.float64) * g64[:, None]).astype(np.float32)

    apply_beta = bool(np.any(beta != 0.0))
    pos_c = np.ascontiguousarray(pos_emb[:S])

    ids32 = input.astype(np.int32)  # [B, S]
    in_maps = []
    for c in range(N_CORES):
        shard = ids32[c * b_per_core:(c + 1) * b_per_core]       # [bpc, S]
        ids_col = np.ascontiguousarray(
            shard.reshape(b_per_core * nt, P).T)                 # [128, bpc*nt]
        m = {
            "ids": ids_col,
            "wemb": word_emb,
            "pos": pos_c,
            "wm": wm_s,
            "wv": wv_s,
        }
        if apply_beta:
            # gamma is folded into the projection weights, so the device
            # kernel computes (xhat + b) @ (gamma*W). Feeding b = beta/gamma
            # makes that equal xhat@(gamma*W) + beta@W, the reference value.
            beta_eff = b64.astype(np.float32)
            m["beta_b"] = np.ascontiguousarray(
                np.broadcast_to(beta_eff, (P, D)).astype(np.float32))
            # A^T bias from the centering: c_h = sum(b) * (colmean - grand)
            c_h = (b64.sum() * (cm[0] - gm)).astype(np.float32)   # [D]
            m["abias"] = np.ascontiguousarray(c_h.reshape(dk, P).T)
        in_maps.append(m)
    return in_maps


def kernel(input, word_emb, pos_emb, gamma, beta, Wk, Wq, Wv):
    input = np.asarray(input)
    B, S = input.shape
    voc, d = np.asarray(word_emb).shape
    assert d == D
    b_per_core = B // N_CORES

    apply_beta = bool(np.any(np.asarray(beta) != 0.0))
    in_maps = prepare_in_maps(input, word_emb, pos_emb, gamma, beta,
                              Wk, Wq, Wv, b_per_core)
    nc = _get_module(b_per_core, S, voc, apply_beta)

    res = run_bass_kernel_spmd(nc, in_maps, core_ids=list(range(N_CORES)))
    out = np.concatenate(
        [r["out"].reshape(b_per_core, S, D) for r in res.results], axis=0)
    return out


# revision 40
# speedup vs baseline: 1.8925x; 1.8925x over previous
"""MiniBert (embed + LayerNorm + single-head attention) on 8 TRN2 NeuronCores.

Strategy: data-parallel over batch (4 sequences per core), embedding table /
pos table / projection weights replicated to every core.

Algebraic fusion: softmax(q k^T) with q = x Wq/sqrt(D), k = x Wk is
softmax(x M x^T) with M = Wq Wk^T/sqrt(D) precomputed on host (gamma folded
in, double-centered — valid because LayerNorm rows are zero-mean) — the K
projection disappears from the device entirely.

Software pipeline (steady state): while sequence b runs attention on the PE
(S = A^T.T @ x^T -> row-max -> exp -> P^T via DMA transpose -> O = P@V), the
next sequence's embedding gathers, LayerNorm (Pool/DVE/ACT), x^T transposes,
V tiles and A^T chunks are woven into the same engine streams so every
engine's in-order queue stays busy and the PE never waits on the softmax
round-trip.
"""
import math
import numpy as np

from concourse import bass, mybir
import concourse.tile as tile
from concourse.bass_utils import run_bass_kernel_spmd
from concourse.masks import make_identity

P = 128
D = 512
VOC = 32000
N_CORES = 8

F32 = mybir.dt.float32
F32R = mybir.dt.float32r
F16 = mybir.dt.float16
I32 = mybir.dt.int32

AF = mybir.ActivationFunctionType
ALU = mybir.AluOpType
AX = mybir.AxisListType


def fix_fat_waits(nc, max_waits=1):
    """Walrus rejects instructions carrying more than ~1 semaphore wait. Tile
    occasionally emits joins (notably the kernel-tail drain) with one wait per
    producing processor. Split the extras into a chain of single-wait NoOps on
    the same engine, inserted immediately before the original instruction."""
    n_new = 0
    for bb in nc.main_func.blocks:
        insts = bb.instructions
        i = 0
        while i < len(insts):
            ins = insts[i]
            si = ins.sync_info
            if si and si.on_wait and len(si.on_wait) > max_waits:
                waits = list(si.on_wait)
                keep = waits[-max_waits:]
                extra = waits[:-max_waits]
                ins.sync_info = mybir.SyncInfo(
                    on_wait=keep, on_update=list(si.on_update or []))
                for j, w in enumerate(extra):
                    nop = mybir.InstNoOp(name=f"W-split-{n_new}", ins=[], outs=[])
                    n_new += 1
                    nop.engine = ins.engine
                    nop.sync_info = mybir.SyncInfo(on_wait=[w], on_update=[])
                    insts.insert(i + j, nop)
                i += len(extra)
            i += 1
    return n_new


def build(b_per_core: int, s_len: int, voc: int, apply_beta: bool,
          debug: bool = False, stages: int = 99,
          bufs_ebuf: int = 2, bufs_sm: int = 3, bufs_psm: int = 2,
          bufs_pss: int = 3):
    """Build the per-core SPMD program. All cores run this same module."""
    nt = s_len // P          # token tiles per sequence (8)
    dk = D // P              # feature tiles (4)
    nchunk = 2               # moving-dim chunks for N=s_len matmuls (512 each)
    ckw = s_len // nchunk    # 512

    nc = bass.Bass()

    ids_d = nc.dram_tensor("ids", [P, b_per_core * nt], I32, kind="ExternalInput")
    wemb_d = nc.dram_tensor("wemb", [voc, D], F32, kind="ExternalInput")
    pos_d = nc.dram_tensor("pos", [s_len, D], F32, kind="ExternalInput")
    wm_d = nc.dram_tensor("wm", [D, D], F32, kind="ExternalInput")
    wv_d = nc.dram_tensor("wv", [D, D], F32, kind="ExternalInput")
    if apply_beta:
        beta_d = nc.dram_tensor("beta_b", [P, D], F32, kind="ExternalInput")
        abias_d = nc.dram_tensor("abias", [P, dk], F32, kind="ExternalInput")
    out_d = nc.dram_tensor("out", [b_per_core * s_len, D], F32, kind="ExternalOutput")

    stages_eff = 99 if stages < 0 else stages

    with tile.TileContext(nc) as tc:
        with (
            tc.tile_pool(name="pers", bufs=1) as pers,
            tc.tile_pool(name="ebuf", bufs=bufs_ebuf) as ebuf,
            tc.tile_pool(name="proj", bufs=2) as proj,
            tc.tile_pool(name="sm", bufs=bufs_sm) as sm,
            tc.tile_pool(name="smt", bufs=4) as smt,
            tc.tile_pool(name="ps_misc", bufs=bufs_psm, space="PSUM") as ps_misc,
            tc.tile_pool(name="ps_s", bufs=bufs_pss, space="PSUM") as ps_s,
        ):
            # ------------- preamble, ordered for the serial DMA pipe -------
            # emission order decides who wins the shared DMA data mover, so:
            # ids -> pos(first half) -> batch-0 gathers -> pos(second half)
            # -> weights. Small gpsimd memsets go first (Pool engine is the
            # gather issuer).
            ident_f = pers.tile([P, P], F32, tag="ident_f")
            make_identity(nc, ident_f[:])
            ident = pers.tile([P, P], F32R, tag="ident")
            nc.vector.tensor_copy(out=ident[:], in_=ident_f[:])
            epsb = pers.tile([P, 1], F32, tag="epsb")
            nc.gpsimd.memset(epsb[:], 1e-5)

            ids_t = pers.tile([P, b_per_core * nt], I32, tag="ids")
            nc.sync.dma_start(out=ids_t[:], in_=ids_d[:, :])

            pos_t = pers.tile([P, nt, D], F32, tag="pos")
            pos_r = pos_d.rearrange("(a p) d -> p a d", p=P)
            nc.sync.dma_start(out=pos_t[:], in_=pos_r[:, :, :])


            # ---- per-batch tile sets ----
            def alloc_tiles(b):
                t = {}
                t["e_all"] = ebuf.tile([P, nt, D], F32, tag="e_all",
                                       name=f"e_all_{b}")
                t["s6"] = ebuf.tile([P, nt, 6], F32, tag="s6", name=f"s6_{b}")
                t["mv"] = ebuf.tile([P, nt, 2], F32, tag="mv", name=f"mv_{b}")
                t["lnv"] = ebuf.tile([P, nt], F32, tag="lnv", name=f"lnv_{b}")
                t["rs"] = ebuf.tile([P, nt], F32, tag="rs", name=f"rs_{b}")
                t["nmurs"] = ebuf.tile([P, nt], F32, tag="nmurs",
                                       name=f"nmurs_{b}")
                t["x_all"] = ebuf.tile([P, nt, D], F32R, tag="x_all",
                                       name=f"x_all_{b}")
                t["xt"] = ebuf.tile([P, dk, s_len], F32R, tag="xt",
                                    name=f"xt_{b}")
                t["at"] = proj.tile([P, dk, s_len], F32R, tag="at",
                                    name=f"at_{b}")
                t["v16"] = proj.tile([P, nt, D], F16, tag="v16",
                                     name=f"v16_{b}")
                t["nm"] = ebuf.tile([P, nt], F32, tag="nm", name=f"nm_{b}")
                t["ls"] = ebuf.tile([P, nt], F32, tag="ls", name=f"ls_{b}")
                t["rr"] = ebuf.tile([P, nt], F32, tag="rr", name=f"rr_{b}")
                t["s_ps"] = [None] * nt
                t["pt16"] = [None] * nt
                return t

            def emit_gathers(b, t):
                if stages_eff < 1:
                    return
                for j in range(nt):
                    nc.gpsimd.indirect_dma_start(
                        out=t["e_all"][:, j, :],
                        out_offset=None,
                        in_=wemb_d[:],
                        in_offset=bass.IndirectOffsetOnAxis(
                            ap=ids_t[:, b * nt + j: b * nt + j + 1], axis=0),
                    )

            def emit_ln(b, t, j, posadd_on_pool):
                """LayerNorm chain for token tile j (no PE work). DVE does
                only DVE-local work (stats + negated mean); the rs-dependent
                products run on ACT so neither engine's in-order queue ever
                blocks on the other's round-trip."""
                if stages_eff < 2:
                    return
                eng = nc.gpsimd if posadd_on_pool else nc.vector
                eng.tensor_tensor(
                    out=t["e_all"][:, j, :], in0=t["e_all"][:, j, :],
                    in1=pos_t[:, j, :], op=ALU.add)
                nc.vector.bn_stats(out=t["s6"][:, j, :], in_=t["e_all"][:, j, :])
                nc.vector.bn_aggr(out=t["mv"][:, j, :], in_=t["s6"][:, j, :])
                # nmu = -mean (DVE-local)
                nc.vector.tensor_scalar_mul(
                    out=t["nmurs"][:, j:j + 1], in0=t["mv"][:, j, 0:1],
                    scalar1=-1.0)
                # rs = exp(-0.5*ln(var + eps)) == rsqrt(var + eps)
                nc.scalar.activation(
                    out=t["lnv"][:, j:j + 1], in_=t["mv"][:, j, 1:2],
                    func=AF.Ln, bias=epsb[:, 0:1], scale=1.0)
                nc.scalar.activation(
                    out=t["rs"][:, j:j + 1], in_=t["lnv"][:, j:j + 1],
                    func=AF.Exp, bias=0.0, scale=-0.5)
                # nmurs = (-mean) * rs, on ACT (scale is a per-partition AP)
                nc.scalar.activation(
                    out=t["nmurs"][:, j:j + 1], in_=t["nmurs"][:, j:j + 1],
                    func=AF.Identity, bias=0.0, scale=t["rs"][:, j:j + 1])
                if apply_beta:
                    xtmp = ebuf.tile([P, D], F32, tag="xtmp",
                                     name=f"xtmp_{b}_{j}")
                    nc.scalar.activation(
                        out=xtmp[:], in_=t["e_all"][:, j, :], func=AF.Identity,
                        bias=t["nmurs"][:, j:j + 1], scale=t["rs"][:, j:j + 1])
                    nc.vector.tensor_tensor(
                        out=t["x_all"][:, j, :], in0=xtmp[:], in1=beta_t[:],
                        op=ALU.add)
                else:
                    nc.scalar.activation(
                        out=t["x_all"][:, j, :], in_=t["e_all"][:, j, :],
                        func=AF.Identity,
                        bias=t["nmurs"][:, j:j + 1], scale=t["rs"][:, j:j + 1])

            def emit_t(b, t, j):
                """x^T transpose for token tile j (PE + DVE copy)."""
                if stages_eff < 3:
                    return
                pst = ps_misc.tile([P, dk, P], F32R, tag="ps_misc",
                                   name=f"pst_{b}_{j}")
                for c in range(dk):
                    nc.tensor.transpose(
                        out=pst[:, c, :],
                        in_=t["x_all"][:, j, c * P:(c + 1) * P],
                        identity=ident[:])
                nc.vector.tensor_copy(
                    out=t["xt"][:, :, j * P:(j + 1) * P], in_=pst[:])

            def emit_v(b, t, j):
                """V tile j = x^T[:, j-window]^T @ Wv' (PE work). Emitted at
                least one unit after emit_t(j) so the xt copy has slack."""
                if stages_eff < 4:
                    return
                psv = ps_misc.tile([P, D], F32, tag="ps_misc",
                                   name=f"psv_{b}_{j}")
                for di in range(dk):
                    nc.tensor.matmul(
                        out=psv[:],
                        lhsT=t["xt"][:, di, j * P:(j + 1) * P],
                        rhs=wv_r[:, di, :],
                        start=(di == 0), stop=(di == dk - 1))
                nc.scalar.copy(out=t["v16"][:, j, :], in_=psv[:])

            def emit_a_chunk(b, t, ch):
                """A^T[:, :, ch-window] = sum_di M'[di]^T @ x^T[di]."""
                if stages_eff < 4:
                    return
                for dj in range(dk):
                    ps = ps_misc.tile([P, ckw], F32, tag="ps_misc",
                                      name=f"psa_{b}_{ch}_{dj}")
                    for di in range(dk):
                        nc.tensor.matmul(
                            out=ps[:],
                            lhsT=wm_r[:, di, dj * P:(dj + 1) * P],
                            rhs=t["xt"][:, di, ch * ckw:(ch + 1) * ckw],
                            start=(di == 0), stop=(di == dk - 1))
                    dst = t["at"][:, dj, ch * ckw:(ch + 1) * ckw]
                    if apply_beta:
                        nc.scalar.activation(
                            out=dst, in_=ps[:], func=AF.Identity,
                            bias=abias_t[:, dj:dj + 1], scale=1.0)
                    else:
                        nc.scalar.copy(out=dst, in_=ps[:])

            # ---- attention stages ----
            def attn_a(b, t, j):
                if stages_eff < 5:
                    return
                s_ps = ps_s.tile([P, nchunk, 512], F32, tag="s_ps",
                                 name=f"sps_{b}_{j}")
                t["s_ps"][j] = s_ps
                for h in range(dk):
                    for ch in range(nchunk):
                        nc.tensor.matmul(
                            out=s_ps[:, ch, :ckw],
                            lhsT=t["at"][:, h, j * P:(j + 1) * P],
                            rhs=t["xt"][:, h, ch * ckw:(ch + 1) * ckw],
                            start=(h == 0), stop=(h == dk - 1))

            def attn_b(b, t, j):
                if stages_eff < 5:
                    return
                s_ps = t["s_ps"][j]
                nc.vector.tensor_reduce(
                    out=t["nm"][:, j:j + 1], in_=s_ps[:, :, :ckw], axis=AX.XY,
                    op=ALU.max, negate=True)
                p16 = sm.tile([P, s_len], F16, tag="p16", name=f"p16_{b}_{j}")
                nc.scalar.activation(
                    out=p16[:].rearrange("p (a d) -> p a d", a=nchunk),
                    in_=s_ps[:, :, :ckw], func=AF.Exp,
                    bias=t["nm"][:, j:j + 1], scale=1.0,
                    accum_out=t["ls"][:, j:j + 1])
                nc.vector.reciprocal(
                    out=t["rr"][:, j:j + 1], in_=t["ls"][:, j:j + 1])
                pt16 = smt.tile([P, nt, P], F16, tag="pt16",
                                name=f"pt16_{b}_{j}")
                t["pt16"][j] = pt16
                nc.sync.dma_start_transpose(pt16[:], p16[:])

            def attn_c(b, t, j):
                if stages_eff < 6:
                    return
                pt16 = t["pt16"][j]
                o_ps = ps_misc.tile([P, D], F32, tag="ps_misc",
                                    name=f"ops_{b}_{j}")
                for k in range(nt):
                    nc.tensor.matmul(
                        out=o_ps[:],
                        lhsT=pt16[:, k, :],
                        rhs=t["v16"][:, k, :],
                        start=(k == 0), stop=(k == nt - 1))
                o_sb = sm.tile([P, D], F32, tag="o_sb", name=f"osb_{b}_{j}")
                nc.vector.tensor_scalar_mul(
                    out=o_sb[:], in0=o_ps[:], scalar1=t["rr"][:, j:j + 1])
                row = (b * nt + j) * P
                nc.sync.dma_start(out=out_d[row:row + P, :], in_=o_sb[:])

            def attn_interleaved(b, t, t_next):
                """Attention for batch b with the next batch's prep units
                woven in. Per q-tile step: S(j), softmax(j), PV(j-2), plus
                one LN unit and (late steps) a transpose or V unit of b+1."""
                if stages_eff < 5:
                    # no attention: still need next batch's prep emitted
                    if t_next is not None:
                        for j in range(nt):
                            emit_ln(b + 1, t_next, j, posadd_on_pool=True)
                            emit_t(b + 1, t_next, j)
                            if j > 0:
                                emit_v(b + 1, t_next, j - 1)
                            for ch in range(nchunk):
                                if j == (ch + 1) * (nt // nchunk) - 1:
                                    emit_a_chunk(b + 1, t_next, ch)
                        emit_v(b + 1, t_next, nt - 1)
                    return
                if t_next is None or nt != 8:
                    # no prep fill (or non-tuned shape): stagger PV behind S
                    # so the softmax+transpose round-trip stays covered by
                    # queued S work.
                    stag = min(4, nt - 1)
                    for j in range(nt):
                        attn_a(b, t, j)
                        attn_b(b, t, j)
                        if j >= stag:
                            attn_c(b, t, j - stag)
                    for j in range(nt - stag, nt):
                        attn_c(b, t, j)
                    if t_next is not None:
                        for j in range(nt):
                            emit_ln(b + 1, t_next, j, posadd_on_pool=True)
                            emit_t(b + 1, t_next, j)
                            if j > 0:
                                emit_v(b + 1, t_next, j - 1)
                            for ch in range(nchunk):
                                if j == (ch + 1) * (nt // nchunk) - 1:
                                    emit_a_chunk(b + 1, t_next, ch)
                        emit_v(b + 1, t_next, nt - 1)
                    return
                attn_a(b, t, 0)
                attn_b(b, t, 0)
                attn_a(b, t, 1)
                attn_b(b, t, 1)
                for j in range(2, nt):
                    attn_a(b, t, j)
                    attn_b(b, t, j)
                    attn_c(b, t, j - 2)
                    emit_ln(b + 1, t_next, j - 2, posadd_on_pool=True)
                    if j >= 4:
                        emit_t(b + 1, t_next, j - 4)
                    if j >= 5:
                        emit_v(b + 1, t_next, j - 5)
                # tail: remaining prep of b+1 covers the last softmax
                # round-trips (the deferred PV stages need ~5us of queued PE
                # work each to hide the exp -> P^T-transpose chain).
                emit_ln(b + 1, t_next, 6, posadd_on_pool=True)
                emit_t(b + 1, t_next, 4)
                emit_v(b + 1, t_next, 3)
                emit_ln(b + 1, t_next, 7, posadd_on_pool=True)
                emit_t(b + 1, t_next, 5)
                emit_v(b + 1, t_next, 4)
                attn_c(b, t, nt - 2)
                emit_a_chunk(b + 1, t_next, 0)
                emit_t(b + 1, t_next, 6)
                emit_v(b + 1, t_next, 5)
                attn_c(b, t, nt - 1)
                emit_t(b + 1, t_next, 7)
                emit_v(b + 1, t_next, 6)
                emit_a_chunk(b + 1, t_next, 1)
                emit_v(b + 1, t_next, 7)

            # ------------- batch 0 prep (cold start) -------------
            wv_r = wm_r = beta_t = abias_t = None

            def emit_beta():
                nonlocal beta_t, abias_t
                if apply_beta:
                    beta_t = pers.tile([P, D], F32, tag="betab", name="betab")
                    nc.sync.dma_start(out=beta_t[:], in_=beta_d[:, :])
                    abias_t = pers.tile([P, dk], F32, tag="abias",
                                        name="abias")
                    nc.sync.dma_start(out=abias_t[:], in_=abias_d[:, :])

            def emit_cvts(gate=None):
                # f32 -> f32r needs a rounding compute op (BIR verifier
                # rejects DMA-fed f32r). Stage through one shared buffer;
                # the copies run on the Pool engine after its gather
                # descriptor generation, off every critical chain. `gate`
                # (cold-start e_all tile): tiny marker DMAs ahead of the
                # weight loads on the same SP queue make them WAIT on late
                # gathers, so the serial DMA data mover serves the
                # LayerNorm-critical gather tiles before the bulky weights.
                nonlocal wv_r, wm_r
                w_st = pers.tile([P, dk, D], F32, tag="w_st", name="w_st")
                wv_r = pers.tile([P, dk, D], F32R, tag="wvr", name="wvr")
                if gate is not None:
                    # marker writes INTO the staging buffer give the weight
                    # DMAs a real WAW dependency on late gather tiles (the
                    # scheduler cannot reorder those away), so the pipe
                    # serves the gathers first. The weight DMA overwrites
                    # the marker bytes immediately after.
                    nc.sync.dma_start(out=w_st[:, 0, 0:1],
                                      in_=gate[:, 5, 0:1])
                nc.sync.dma_start(
                    out=w_st[:], in_=wv_d.rearrange("(a p) n -> p a n", p=P))
                nc.gpsimd.tensor_copy(out=wv_r[:], in_=w_st[:])
                wm_r = pers.tile([P, dk, D], F32R, tag="wmr", name="wmr")
                if gate is not None:
                    nc.sync.dma_start(out=w_st[:, 0, 0:1],
                                      in_=gate[:, 7, 0:1])
                nc.sync.dma_start(
                    out=w_st[:], in_=wm_d.rearrange("(a p) n -> p a n", p=P))
                nc.gpsimd.tensor_copy(out=wm_r[:], in_=w_st[:])

            tiles = [None] * max(b_per_core, 1)
            if stages < 0:
                # hardware-loop variant (unused by the test harness's timing
                # path for stages>=0): simple non-pipelined loop
                emit_beta()
                emit_cvts()
                with tc.For_i(0, -stages, 1):
                    t0 = alloc_tiles(0)
                    emit_gathers(0, t0)
                    for j in range(nt):
                        emit_ln(0, t0, j, posadd_on_pool=False)
                        emit_t(0, t0, j)
                        if j > 0:
                            emit_v(0, t0, j - 1)
                        for ch in range(nchunk):
                            if j == (ch + 1) * (nt // nchunk) - 1:
                                emit_a_chunk(0, t0, ch)
                    emit_v(0, t0, nt - 1)
                    attn_interleaved(0, t0, None)
            else:
                emit_beta()
                tiles[0] = alloc_tiles(0)
                emit_gathers(0, tiles[0])
                emit_cvts(gate=tiles[0]["e_all"]
                          if nt == 8 and stages_eff >= 1 else None)

                # cold start, ordered to match DMA arrivals: early token
                # tiles transpose as they land; V tiles interleave once the
                # (gather-gated) Wv arrives; A^T chunks once Wm lands.
                t0 = tiles[0]
                if nt == 8:
                    for j in range(nt):
                        emit_ln(0, t0, j, posadd_on_pool=False)
                    for j in range(4):
                        emit_t(0, t0, j)
                    emit_t(0, t0, 4)
                    emit_v(0, t0, 0)
                    emit_t(0, t0, 5)
                    emit_v(0, t0, 1)
                    emit_v(0, t0, 2)
                    emit_v(0, t0, 3)
                    emit_a_chunk(0, t0, 0)
                    emit_t(0, t0, 6)
                    emit_v(0, t0, 4)
                    emit_t(0, t0, 7)
                    emit_v(0, t0, 5)
                    emit_v(0, t0, 6)
                    emit_v(0, t0, 7)
                    emit_a_chunk(0, t0, 1)
                else:
                    for j in range(nt):
                        emit_ln(0, t0, j, posadd_on_pool=False)
                        emit_t(0, t0, j)
                    for j in range(nt):
                        emit_v(0, t0, j)
                        for ch in range(nchunk):
                            if j == (ch + 1) * (nt // nchunk) - 1:
                                emit_a_chunk(0, t0, ch)
                first_attn = 0

                for b in range(first_attn, b_per_core):
                    nxt = b + 1 if b + 1 < b_per_core else None
                    if nxt is not None:
                        if tiles[nxt] is None:
                            tiles[nxt] = alloc_tiles(nxt)
                            emit_gathers(nxt, tiles[nxt])
                        attn_interleaved(b, tiles[b], tiles[nxt])
                    else:
                        attn_interleaved(b, tiles[b], None)

    fix_fat_waits(nc)
    return nc


_CACHE = {}


def _get_module(b_per_core, s_len, voc, apply_beta, stages=99):
    key = (b_per_core, s_len, voc, apply_beta, stages)
    if key not in _CACHE:
        _CACHE[key] = build(b_per_core, s_len, voc, apply_beta, stages=stages)
    return _CACHE[key]


def prepare_in_maps(input, word_emb, pos_emb, gamma, beta, Wk, Wq, Wv,
                    b_per_core):
    """Host-side preprocessing -> per-core input maps (one dict per core)."""
    input = np.asarray(input)
    word_emb = np.ascontiguousarray(np.asarray(word_emb, dtype=np.float32))
    pos_emb = np.asarray(pos_emb, dtype=np.float32)
    gamma = np.asarray(gamma, dtype=np.float32)
    beta = np.asarray(beta, dtype=np.float32)
    Wk = np.asarray(Wk, dtype=np.float32)
    Wq = np.asarray(Wq, dtype=np.float32)
    Wv = np.asarray(Wv, dtype=np.float32)

    B, S = input.shape
    nt = S // P
    dk = D // P

    # Fused logit matrix M = (g*Wq)(g*Wk)^T / sqrt(D): softmax(q k^T) ==
    # softmax(x_dev M x_dev^T). Double-center M (rows of x_dev sum to the
    # host-known constant sum(beta/gamma); the LayerNorm part sums to zero)
    # to keep fp32r matmul operands small and well-conditioned.
    g64 = gamma.astype(np.float64)
    b64 = beta.astype(np.float64) / np.where(g64 == 0.0, 1.0, g64)
    wq64 = Wq.astype(np.float64) * g64[:, None]
    wk64 = Wk.astype(np.float64) * g64[:, None]
    m64 = wq64 @ wk64.T / math.sqrt(D)
    cm = m64.mean(0, keepdims=True)
    rm = m64.mean(1, keepdims=True)
    gm = m64.mean()
    wm_s = np.ascontiguousarray((m64 - cm - rm + gm).astype(np.float32))
    wv_s = (Wv.astype(np.float64) * g64[:, None]).astype(np.float32)

    apply_beta = bool(np.any(beta != 0.0))
    pos_c = np.ascontiguousarray(pos_emb[:S])

    ids32 = input.astype(np.int32)  # [B, S]
    in_maps = []
    for c in range(N_CORES):
        shard = ids32[c * b_per_core:(c + 1) * b_per_core]       # [bpc, S]
        ids_col = np.ascontiguousarray(
            shard.reshape(b_per_core * nt, P).T)                 # [128, bpc*nt]
        m = {
            "ids": ids_col,
            "wemb": word_emb,
            "pos": pos_c,
            "wm": wm_s,
            "wv": wv_s,
        }
        if apply_beta:
            # gamma is folded into the projection weights, so the device
            # kernel computes (xhat + b) @ (gamma*W). Feeding b = beta/gamma
            # makes that equal xhat@(gamma*W) + beta@W, the reference value.
            beta_eff = b64.astype(np.float32)
            m["beta_b"] = np.ascontiguousarray(
                np.broadcast_to(beta_eff, (P, D)).astype(np.float32))
            # A^T bias from the centering: c_h = sum(b) * (colmean - grand)
            c_h = (b64.sum() * (cm[0] - gm)).astype(np.float32)   # [D]
            m["abias"] = np.ascontiguousarray(c_h.reshape(dk, P).T)
        in_maps.append(m)
    return in_maps


def kernel(input, word_emb, pos_emb, gamma, beta, Wk, Wq, Wv):
    input = np.asarray(input)
    B, S = input.shape
    voc, d = np.asarray(word_emb).shape
    assert d == D
    b_per_core = B // N_CORES

    apply_beta = bool(np.any(np.asarray(beta) != 0.0))
    in_maps = prepare_in_maps(input, word_emb, pos_emb, gamma, beta,
                              Wk, Wq, Wv, b_per_core)
    nc = _get_module(b_per_core, S, voc, apply_beta)

    res = run_bass_kernel_spmd(nc, in_maps, core_ids=list(range(N_CORES)))
    out = np.concatenate(
        [r["out"].reshape(b_per_core, S, D) for r in res.results], axis=0)
    return out


# revision 49
# speedup vs baseline: 2.1459x; 1.1339x over previous
"""MiniBert (embed + LayerNorm + single-head attention) on 8 TRN2 NeuronCores.

Strategy: data-parallel over batch (4 sequences per core), embedding table /
pos table / projection weights replicated to every core.

Algebraic fusion: softmax(q k^T) with q = x Wq/sqrt(D), k = x Wk is
softmax(x M x^T) with M = Wq Wk^T/sqrt(D) precomputed on host (gamma folded
in, double-centered — valid because LayerNorm rows are zero-mean) — the K
projection disappears from the device entirely.

Software pipeline (steady state): while sequence b runs attention on the PE
(S = A^T.T @ x^T -> row-max -> exp -> P^T via DMA transpose -> O = P@V), the
next sequence's embedding gathers, LayerNorm (Pool/DVE/ACT), x^T transposes,
V tiles and A^T chunks are woven into the same engine streams so every
engine's in-order queue stays busy and the PE never waits on the softmax
round-trip.
"""
import math
import numpy as np

from concourse import bass, mybir
import concourse.tile as tile
from concourse.bass_utils import run_bass_kernel_spmd
from concourse.masks import make_identity

P = 128
D = 512
VOC = 32000
N_CORES = 8

F32 = mybir.dt.float32
F32R = mybir.dt.float32r
F16 = mybir.dt.float16
I32 = mybir.dt.int32

AF = mybir.ActivationFunctionType
ALU = mybir.AluOpType
AX = mybir.AxisListType


def fix_fat_waits(nc, max_waits=1):
    """Walrus rejects instructions carrying more than ~1 semaphore wait. Tile
    occasionally emits joins (notably the kernel-tail drain) with one wait per
    producing processor. Split the extras into a chain of single-wait NoOps on
    the same engine, inserted immediately before the original instruction."""
    n_new = 0
    for bb in nc.main_func.blocks:
        insts = bb.instructions
        i = 0
        while i < len(insts):
            ins = insts[i]
            si = ins.sync_info
            if si and si.on_wait and len(si.on_wait) > max_waits:
                waits = list(si.on_wait)
                keep = waits[-max_waits:]
                extra = waits[:-max_waits]
                ins.sync_info = mybir.SyncInfo(
                    on_wait=keep, on_update=list(si.on_update or []))
                for j, w in enumerate(extra):
                    nop = mybir.InstNoOp(name=f"W-split-{n_new}", ins=[], outs=[])
                    n_new += 1
                    nop.engine = ins.engine
                    nop.sync_info = mybir.SyncInfo(on_wait=[w], on_update=[])
                    insts.insert(i + j, nop)
                i += len(extra)
            i += 1
    return n_new


def build(b_per_core: int, s_len: int, voc: int, apply_beta: bool,
          debug: bool = False, stages: int = 99,
          bufs_ebuf: int = 2, bufs_sm: int = 3, bufs_psm: int = 2,
          bufs_pss: int = 3):
    """Build the per-core SPMD program. All cores run this same module."""
    nt = s_len // P          # token tiles per sequence (8)
    dk = D // P              # feature tiles (4)
    nchunk = 2               # moving-dim chunks for N=s_len matmuls (512 each)
    ckw = s_len // nchunk    # 512

    nc = bass.Bass()

    ids_d = nc.dram_tensor("ids", [P, b_per_core * nt], I32, kind="ExternalInput")
    wemb_d = nc.dram_tensor("wemb", [voc, D], F32, kind="ExternalInput")
    pos_d = nc.dram_tensor("pos", [s_len, D], F32, kind="ExternalInput")
    wm_d = nc.dram_tensor("wm", [D, D], F32, kind="ExternalInput")
    wv_d = nc.dram_tensor("wv", [D, D], F32, kind="ExternalInput")
    if apply_beta:
        beta_d = nc.dram_tensor("beta_b", [P, D], F32, kind="ExternalInput")
        abias_d = nc.dram_tensor("abias", [P, dk], F32, kind="ExternalInput")
    out_d = nc.dram_tensor("out", [b_per_core * s_len, D], F32, kind="ExternalOutput")

    stages_eff = 99 if stages < 0 else stages

    with tile.TileContext(nc) as tc:
        with (
            tc.tile_pool(name="pers", bufs=1) as pers,
            tc.tile_pool(name="ebuf", bufs=bufs_ebuf) as ebuf,
            tc.tile_pool(name="proj", bufs=2) as proj,
            tc.tile_pool(name="sm", bufs=bufs_sm) as sm,
            tc.tile_pool(name="smt", bufs=4) as smt,
            tc.tile_pool(name="ps_misc", bufs=bufs_psm, space="PSUM") as ps_misc,
            tc.tile_pool(name="ps_s", bufs=bufs_pss, space="PSUM") as ps_s,
        ):
            # ------------- preamble, ordered for the serial DMA pipe -------
            # emission order decides who wins the shared DMA data mover, so:
            # ids -> pos(first half) -> batch-0 gathers -> pos(second half)
            # -> weights. Small gpsimd memsets go first (Pool engine is the
            # gather issuer).
            ident_f = pers.tile([P, P], F32, tag="ident_f")
            make_identity(nc, ident_f[:])
            ident = pers.tile([P, P], F32R, tag="ident")
            nc.vector.tensor_copy(out=ident[:], in_=ident_f[:])
            ident16 = pers.tile([P, P], F16, tag="ident16")
            nc.vector.tensor_copy(out=ident16[:], in_=ident_f[:])
            epsb = pers.tile([P, 1], F32, tag="epsb")
            nc.gpsimd.memset(epsb[:], 1e-5)

            ids_t = pers.tile([P, b_per_core * nt], I32, tag="ids")
            nc.sync.dma_start(out=ids_t[:], in_=ids_d[:, :])

            pos_t = pers.tile([P, nt, D], F32, tag="pos")
            pos_r = pos_d.rearrange("(a p) d -> p a d", p=P)
            nc.sync.dma_start(out=pos_t[:], in_=pos_r[:, :, :])


            # ---- per-batch tile sets ----
            def alloc_tiles(b):
                t = {}
                t["e_all"] = ebuf.tile([P, nt, D], F32, tag="e_all",
                                       name=f"e_all_{b}")
                t["s6"] = ebuf.tile([P, nt, 6], F32, tag="s6", name=f"s6_{b}")
                t["mv"] = ebuf.tile([P, nt, 2], F32, tag="mv", name=f"mv_{b}")
                t["lnv"] = ebuf.tile([P, nt], F32, tag="lnv", name=f"lnv_{b}")
                t["rs"] = ebuf.tile([P, nt], F32, tag="rs", name=f"rs_{b}")
                t["nmurs"] = ebuf.tile([P, nt], F32, tag="nmurs",
                                       name=f"nmurs_{b}")
                t["x_all"] = ebuf.tile([P, nt, D], F32R, tag="x_all",
                                       name=f"x_all_{b}")
                t["xt"] = ebuf.tile([P, dk, s_len], F32R, tag="xt",
                                    name=f"xt_{b}")
                t["at"] = proj.tile([P, dk, s_len], F32R, tag="at",
                                    name=f"at_{b}")
                t["v16"] = proj.tile([P, nt, D], F16, tag="v16",
                                     name=f"v16_{b}")
                t["nm"] = ebuf.tile([P, nt], F32, tag="nm", name=f"nm_{b}")
                t["ls"] = ebuf.tile([P, nt], F32, tag="ls", name=f"ls_{b}")
                t["rr"] = ebuf.tile([P, nt], F32, tag="rr", name=f"rr_{b}")
                t["s_ps"] = [None] * nt
                t["pt16"] = [None] * nt
                t["pt_src"] = [None] * nt
                return t

            def emit_gathers(b, t):
                if stages_eff < 1:
                    return
                for j in range(nt):
                    nc.gpsimd.indirect_dma_start(
                        out=t["e_all"][:, j, :],
                        out_offset=None,
                        in_=wemb_d[:],
                        in_offset=bass.IndirectOffsetOnAxis(
                            ap=ids_t[:, b * nt + j: b * nt + j + 1], axis=0),
                    )

            def emit_ln(b, t, j, posadd_on_pool):
                """LayerNorm chain for token tile j (no PE work). DVE does
                only DVE-local work (stats + negated mean); the rs-dependent
                products run on ACT so neither engine's in-order queue ever
                blocks on the other's round-trip."""
                if stages_eff < 2:
                    return
                eng = nc.gpsimd if posadd_on_pool else nc.vector
                eng.tensor_tensor(
                    out=t["e_all"][:, j, :], in0=t["e_all"][:, j, :],
                    in1=pos_t[:, j, :], op=ALU.add)
                nc.vector.bn_stats(out=t["s6"][:, j, :], in_=t["e_all"][:, j, :])
                nc.vector.bn_aggr(out=t["mv"][:, j, :], in_=t["s6"][:, j, :])
                # nmu = -mean (DVE-local)
                nc.vector.tensor_scalar_mul(
                    out=t["nmurs"][:, j:j + 1], in0=t["mv"][:, j, 0:1],
                    scalar1=-1.0)
                # rs = exp(-0.5*ln(var + eps)) == rsqrt(var + eps)
                nc.scalar.activation(
                    out=t["lnv"][:, j:j + 1], in_=t["mv"][:, j, 1:2],
                    func=AF.Ln, bias=epsb[:, 0:1], scale=1.0)
                nc.scalar.activation(
                    out=t["rs"][:, j:j + 1], in_=t["lnv"][:, j:j + 1],
                    func=AF.Exp, bias=0.0, scale=-0.5)
                # nmurs = (-mean) * rs, on ACT (scale is a per-partition AP)
                nc.scalar.activation(
                    out=t["nmurs"][:, j:j + 1], in_=t["nmurs"][:, j:j + 1],
                    func=AF.Identity, bias=0.0, scale=t["rs"][:, j:j + 1])
                if apply_beta:
                    xtmp = ebuf.tile([P, D], F32, tag="xtmp",
                                     name=f"xtmp_{b}_{j}")
                    nc.scalar.activation(
                        out=xtmp[:], in_=t["e_all"][:, j, :], func=AF.Identity,
                        bias=t["nmurs"][:, j:j + 1], scale=t["rs"][:, j:j + 1])
                    nc.vector.tensor_tensor(
                        out=t["x_all"][:, j, :], in0=xtmp[:], in1=beta_t[:],
                        op=ALU.add)
                else:
                    nc.scalar.activation(
                        out=t["x_all"][:, j, :], in_=t["e_all"][:, j, :],
                        func=AF.Identity,
                        bias=t["nmurs"][:, j:j + 1], scale=t["rs"][:, j:j + 1])

            def emit_t(b, t, j):
                """x^T transpose for token tile j (PE + DVE copy)."""
                if stages_eff < 3:
                    return
                pst = ps_misc.tile([P, dk, P], F32R, tag="ps_misc",
                                   name=f"pst_{b}_{j}")
                for c in range(dk):
                    nc.tensor.transpose(
                        out=pst[:, c, :],
                        in_=t["x_all"][:, j, c * P:(c + 1) * P],
                        identity=ident[:])
                nc.vector.tensor_copy(
                    out=t["xt"][:, :, j * P:(j + 1) * P], in_=pst[:])

            def emit_v(b, t, j):
                """V tile j = x^T[:, j-window]^T @ Wv' (PE work). Emitted at
                least one unit after emit_t(j) so the xt copy has slack."""
                if stages_eff < 4:
                    return
                psv = ps_misc.tile([P, D], F32, tag="ps_misc",
                                   name=f"psv_{b}_{j}")
                for di in range(dk):
                    nc.tensor.matmul(
                        out=psv[:],
                        lhsT=t["xt"][:, di, j * P:(j + 1) * P],
                        rhs=wv_r[:, di, :],
                        start=(di == 0), stop=(di == dk - 1))
                nc.scalar.copy(out=t["v16"][:, j, :], in_=psv[:])

            def emit_a_chunk(b, t, ch):
                """A^T[:, :, ch-window] = sum_di M'[di]^T @ x^T[di]."""
                if stages_eff < 4:
                    return
                for dj in range(dk):
                    ps = ps_misc.tile([P, ckw], F32, tag="ps_misc",
                                      name=f"psa_{b}_{ch}_{dj}")
                    for di in range(dk):
                        nc.tensor.matmul(
                            out=ps[:],
                            lhsT=wm_r[:, di, dj * P:(dj + 1) * P],
                            rhs=t["xt"][:, di, ch * ckw:(ch + 1) * ckw],
                            start=(di == 0), stop=(di == dk - 1))
                    dst = t["at"][:, dj, ch * ckw:(ch + 1) * ckw]
                    if apply_beta:
                        nc.scalar.activation(
                            out=dst, in_=ps[:], func=AF.Identity,
                            bias=abias_t[:, dj:dj + 1], scale=1.0)
                    else:
                        nc.scalar.copy(out=dst, in_=ps[:])

            # ---- attention stages ----
            def attn_a(b, t, j):
                if stages_eff < 5:
                    return
                s_ps = ps_s.tile([P, nchunk, 512], F32, tag="s_ps",
                                 name=f"sps_{b}_{j}")
                t["s_ps"][j] = s_ps
                for h in range(dk):
                    for ch in range(nchunk):
                        nc.tensor.matmul(
                            out=s_ps[:, ch, :ckw],
                            lhsT=t["at"][:, h, j * P:(j + 1) * P],
                            rhs=t["xt"][:, h, ch * ckw:(ch + 1) * ckw],
                            start=(h == 0), stop=(h == dk - 1))

            def attn_b(b, t, j, dma_tr=True):
                if stages_eff < 5:
                    return
                s_ps = t["s_ps"][j]
                nc.vector.tensor_reduce(
                    out=t["nm"][:, j:j + 1], in_=s_ps[:, :, :ckw], axis=AX.XY,
                    op=ALU.max, negate=True)
                p16 = sm.tile([P, s_len], F16, tag="p16", name=f"p16_{b}_{j}")
                nc.scalar.activation(
                    out=p16[:].rearrange("p (a d) -> p a d", a=nchunk),
                    in_=s_ps[:, :, :ckw], func=AF.Exp,
                    bias=t["nm"][:, j:j + 1], scale=1.0,
                    accum_out=t["ls"][:, j:j + 1])
                nc.vector.reciprocal(
                    out=t["rr"][:, j:j + 1], in_=t["ls"][:, j:j + 1])
                t["pt_src"][j] = p16
                if dma_tr:
                    pt16 = smt.tile([P, nt, P], F16, tag="pt16",
                                    name=f"pt16_{b}_{j}")
                    t["pt16"][j] = pt16
                    nc.sync.dma_start_transpose(pt16[:], p16[:])

            def attn_pt_pe(b, t, j):
                """P^T via PE transposes (f16 through PSUM) + DVE copy.
                Shorter latency than the DMA transpose; used for the final
                q-tiles of the last sequence where no fill work remains."""
                p16 = t["pt_src"][j]
                ps_t = ps_misc.tile([P, nt, P], F16, tag="ps_misc",
                                    name=f"pstr_{b}_{j}")
                for k in range(nt):
                    nc.tensor.transpose(
                        out=ps_t[:, k, :], in_=p16[:, k * P:(k + 1) * P],
                        identity=ident16[:])
                pt16 = smt.tile([P, nt, P], F16, tag="pt16",
                                name=f"pt16_{b}_{j}")
                t["pt16"][j] = pt16
                nc.vector.tensor_copy(out=pt16[:], in_=ps_t[:])

            def attn_c(b, t, j):
                if stages_eff < 6:
                    return
                pt16 = t["pt16"][j]
                o_ps = ps_misc.tile([P, D], F32, tag="ps_misc",
                                    name=f"ops_{b}_{j}")
                for k in range(nt):
                    nc.tensor.matmul(
                        out=o_ps[:],
                        lhsT=pt16[:, k, :],
                        rhs=t["v16"][:, k, :],
                        start=(k == 0), stop=(k == nt - 1))
                o_sb = sm.tile([P, D], F32, tag="o_sb", name=f"osb_{b}_{j}")
                nc.vector.tensor_scalar_mul(
                    out=o_sb[:], in0=o_ps[:], scalar1=t["rr"][:, j:j + 1])
                row = (b * nt + j) * P
                nc.sync.dma_start(out=out_d[row:row + P, :], in_=o_sb[:])

            def attn_interleaved(b, t, t_next):
                """Attention for batch b with the next batch's prep units
                woven in. Per q-tile step: S(j), softmax(j), PV(j-2), plus
                one LN unit and (late steps) a transpose or V unit of b+1."""
                if stages_eff < 5:
                    # no attention: still need next batch's prep emitted
                    if t_next is not None:
                        for j in range(nt):
                            emit_ln(b + 1, t_next, j, posadd_on_pool=True)
                            emit_t(b + 1, t_next, j)
                            if j > 0:
                                emit_v(b + 1, t_next, j - 1)
                            for ch in range(nchunk):
                                if j == (ch + 1) * (nt // nchunk) - 1:
                                    emit_a_chunk(b + 1, t_next, ch)
                        emit_v(b + 1, t_next, nt - 1)
                    return
                if t_next is None or nt != 8:
                    # no prep fill (or non-tuned shape): stagger PV behind S
                    # so the softmax+transpose round-trip stays covered by
                    # queued S work. The final two P^T transposes run on the
                    # PE (short chain) since nothing is left to hide the DMA
                    # round-trip behind.
                    stag = min(4, nt - 1)
                    pe_tr = set([nt - 2, nt - 1]) if (
                        nt == 8 and t_next is None) else set()
                    for j in range(nt):
                        attn_a(b, t, j)
                        attn_b(b, t, j, dma_tr=j not in pe_tr)
                        if j >= stag:
                            attn_c(b, t, j - stag)
                    tail = list(range(nt - stag, nt))
                    for i, j in enumerate(tail):
                        if nt - 2 in pe_tr and j == tail[0]:
                            attn_pt_pe(b, t, nt - 2)
                        if nt - 1 in pe_tr and j == tail[1 % len(tail)]:
                            attn_pt_pe(b, t, nt - 1)
                        attn_c(b, t, j)
                    if t_next is not None:
                        for j in range(nt):
                            emit_ln(b + 1, t_next, j, posadd_on_pool=True)
                            emit_t(b + 1, t_next, j)
                            if j > 0:
                                emit_v(b + 1, t_next, j - 1)
                            for ch in range(nchunk):
                                if j == (ch + 1) * (nt // nchunk) - 1:
                                    emit_a_chunk(b + 1, t_next, ch)
                        emit_v(b + 1, t_next, nt - 1)
                    return
                attn_a(b, t, 0)
                attn_b(b, t, 0)
                attn_a(b, t, 1)
                attn_b(b, t, 1)
                for j in range(2, nt):
                    attn_a(b, t, j)
                    attn_b(b, t, j)
                    attn_c(b, t, j - 2)
                    emit_ln(b + 1, t_next, j - 2, posadd_on_pool=True)
                    if j >= 4:
                        emit_t(b + 1, t_next, j - 4)
                    if j >= 5:
                        emit_v(b + 1, t_next, j - 5)
                # tail: remaining prep of b+1 covers the last softmax
                # round-trips (the deferred PV stages need ~5us of queued PE
                # work each to hide the exp -> P^T-transpose chain).
                emit_ln(b + 1, t_next, 6, posadd_on_pool=True)
                emit_t(b + 1, t_next, 4)
                emit_v(b + 1, t_next, 3)
                emit_ln(b + 1, t_next, 7, posadd_on_pool=True)
                emit_t(b + 1, t_next, 5)
                emit_v(b + 1, t_next, 4)
                emit_a_chunk(b + 1, t_next, 0)
                attn_c(b, t, nt - 2)
                emit_t(b + 1, t_next, 6)
                emit_v(b + 1, t_next, 5)
                emit_t(b + 1, t_next, 7)
                emit_v(b + 1, t_next, 6)
                attn_c(b, t, nt - 1)
                emit_a_chunk(b + 1, t_next, 1)
                emit_v(b + 1, t_next, 7)

            # ------------- batch 0 prep (cold start) -------------
            wv_r = wm_r = beta_t = abias_t = None

            def emit_beta():
                nonlocal beta_t, abias_t
                if apply_beta:
                    beta_t = pers.tile([P, D], F32, tag="betab", name="betab")
                    nc.sync.dma_start(out=beta_t[:], in_=beta_d[:, :])
                    abias_t = pers.tile([P, dk], F32, tag="abias",
                                        name="abias")
                    nc.sync.dma_start(out=abias_t[:], in_=abias_d[:, :])

            def emit_cvts(gate=None):
                # f32 -> f32r needs a rounding compute op (BIR verifier
                # rejects DMA-fed f32r). Stage through one shared buffer;
                # the copies run on the Pool engine after its gather
                # descriptor generation, off every critical chain. `gate`
                # (cold-start e_all tile): tiny marker DMAs ahead of the
                # weight loads on the same SP queue make them WAIT on late
                # gathers, so the serial DMA data mover serves the
                # LayerNorm-critical gather tiles before the bulky weights.
                nonlocal wv_r, wm_r
                w_st = pers.tile([P, dk, D], F32, tag="w_st", name="w_st")
                wv_r = pers.tile([P, dk, D], F32R, tag="wvr", name="wvr")
                if gate is not None:
                    # marker writes INTO the staging buffer give the weight
                    # DMAs a real WAW dependency on late gather tiles (the
                    # scheduler cannot reorder those away), so the pipe
                    # serves the gathers first. The weight DMA overwrites
                    # the marker bytes immediately after.
                    nc.sync.dma_start(out=w_st[:, 0, 0:1],
                                      in_=gate[:, 5, 0:1])
                nc.sync.dma_start(
                    out=w_st[:], in_=wv_d.rearrange("(a p) n -> p a n", p=P))
                nc.gpsimd.tensor_copy(out=wv_r[:], in_=w_st[:])
                wm_r = pers.tile([P, dk, D], F32R, tag="wmr", name="wmr")
                if gate is not None:
                    nc.sync.dma_start(out=w_st[:, 0, 0:1],
                                      in_=gate[:, 7, 0:1])
                nc.sync.dma_start(
                    out=w_st[:], in_=wm_d.rearrange("(a p) n -> p a n", p=P))
                nc.gpsimd.tensor_copy(out=wm_r[:], in_=w_st[:])
                return w_st

            def emit_w_chunk(w_st, j):
                pass

            tiles = [None] * max(b_per_core, 1)
            if stages < 0:
                # hardware-loop variant (unused by the test harness's timing
                # path for stages>=0): simple non-pipelined loop
                emit_beta()
                emit_cvts()
                with tc.For_i(0, -stages, 1):
                    t0 = alloc_tiles(0)
                    emit_gathers(0, t0)
                    for j in range(nt):
                        emit_ln(0, t0, j, posadd_on_pool=False)
                        emit_t(0, t0, j)
                        if j > 0:
                            emit_v(0, t0, j - 1)
                        for ch in range(nchunk):
                            if j == (ch + 1) * (nt // nchunk) - 1:
                                emit_a_chunk(0, t0, ch)
                    emit_v(0, t0, nt - 1)
                    attn_interleaved(0, t0, None)
            else:
                emit_beta()
                tiles[0] = alloc_tiles(0)
                emit_gathers(0, tiles[0])
                use_gate = nt == 8 and stages_eff >= 1
                w_st = emit_cvts(
                    gate=tiles[0]["e_all"] if use_gate else None)

                # cold start, ordered to match DMA arrivals: early token
                # tiles transpose as they land; V tiles interleave once the
                # (gather-gated) Wv arrives; A^T chunks once Wm lands.
                t0 = tiles[0]
                if nt == 8:
                    for j in range(nt):
                        emit_ln(0, t0, j, posadd_on_pool=False)
                        emit_w_chunk(w_st, j)
                    for j in range(4):
                        emit_t(0, t0, j)
                    emit_t(0, t0, 4)
                    emit_v(0, t0, 0)
                    emit_t(0, t0, 5)
                    emit_v(0, t0, 1)
                    emit_v(0, t0, 2)
                    emit_v(0, t0, 3)
                    emit_a_chunk(0, t0, 0)
                    emit_t(0, t0, 6)
                    emit_v(0, t0, 4)
                    emit_t(0, t0, 7)
                    emit_v(0, t0, 5)
                    emit_v(0, t0, 6)
                    emit_v(0, t0, 7)
                    emit_a_chunk(0, t0, 1)
                else:
                    for j in range(nt):
                        emit_ln(0, t0, j, posadd_on_pool=False)
                        if use_gate:
                            emit_w_chunk(w_st, j)
                        emit_t(0, t0, j)
                    for j in range(nt):
                        emit_v(0, t0, j)
                        for ch in range(nchunk):
                            if j == (ch + 1) * (nt // nchunk) - 1:
                                emit_a_chunk(0, t0, ch)
                first_attn = 0

                for b in range(first_attn, b_per_core):
                    nxt = b + 1 if b + 1 < b_per_core else None
                    if nxt is not None:
                        if tiles[nxt] is None:
                            tiles[nxt] = alloc_tiles(nxt)
                            emit_gathers(nxt, tiles[nxt])
                        attn_interleaved(b, tiles[b], tiles[nxt])
                    else:
                        attn_interleaved(b, tiles[b], None)

    fix_fat_waits(nc)
    return nc


_CACHE = {}


def _get_module(b_per_core, s_len, voc, apply_beta, stages=99):
    key = (b_per_core, s_len, voc, apply_beta, stages)
    if key not in _CACHE:
        _CACHE[key] = build(b_per_core, s_len, voc, apply_beta, stages=stages)
    return _CACHE[key]


def prepare_in_maps(input, word_emb, pos_emb, gamma, beta, Wk, Wq, Wv,
                    b_per_core):
    """Host-side preprocessing -> per-core input maps (one dict per core)."""
    input = np.asarray(input)
    word_emb = np.ascontiguousarray(np.asarray(word_emb, dtype=np.float32))
    pos_emb = np.asarray(pos_emb, dtype=np.float32)
    gamma = np.asarray(gamma, dtype=np.float32)
    beta = np.asarray(beta, dtype=np.float32)
    Wk = np.asarray(Wk, dtype=np.float32)
    Wq = np.asarray(Wq, dtype=np.float32)
    Wv = np.asarray(Wv, dtype=np.float32)

    B, S = input.shape
    nt = S // P
    dk = D // P

    # Fused logit matrix M = (g*Wq)(g*Wk)^T / sqrt(D): softmax(q k^T) ==
    # softmax(x_dev M x_dev^T). Double-center M (rows of x_dev sum to the
    # host-known constant sum(beta/gamma); the LayerNorm part sums to zero)
    # to keep fp32r matmul operands small and well-conditioned.
    g64 = gamma.astype(np.float64)
    b64 = beta.astype(np.float64) / np.where(g64 == 0.0, 1.0, g64)
    wq64 = Wq.astype(np.float64) * g64[:, None]
    wk64 = Wk.astype(np.float64) * g64[:, None]
    m64 = wq64 @ wk64.T / math.sqrt(D)
    cm = m64.mean(0, keepdims=True)
    rm = m64.mean(1, keepdims=True)
    gm = m64.mean()
    wm_s = np.ascontiguousarray((m64 - cm - rm + gm).astype(np.float32))
    wv_s = (Wv.astype(np.float64) * g64[:, None]).astype(np.float32)

    apply_beta = bool(np.any(beta != 0.0))
    pos_c = np.ascontiguousarray(pos_emb[:S])

    ids32 = input.astype(np.int32)  # [B, S]
    in_maps = []
    for c in range(N_CORES):
        shard = ids32[c * b_per_core:(c + 1) * b_per_core]       # [bpc, S]
        ids_col = np.ascontiguousarray(
            shard.reshape(b_per_core * nt, P).T)                 # [128, bpc*nt]
        m = {
            "ids": ids_col,
            "wemb": word_emb,
            "pos": pos_c,
            "wm": wm_s,
            "wv": wv_s,
        }
        if apply_beta:
            # gamma is folded into the projection weights, so the device
            # kernel computes (xhat + b) @ (gamma*W). Feeding b = beta/gamma
            # makes that equal xhat@(gamma*W) + beta@W, the reference value.
            beta_eff = b64.astype(np.float32)
            m["beta_b"] = np.ascontiguousarray(
                np.broadcast_to(beta_eff, (P, D)).astype(np.float32))
            # A^T bias from the centering: c_h = sum(b) * (colmean - grand)
            c_h = (b64.sum() * (cm[0] - gm)).astype(np.float32)   # [D]
            m["abias"] = np.ascontiguousarray(c_h.reshape(dk, P).T)
        in_maps.append(m)
    return in_maps


def kernel(input, word_emb, pos_emb, gamma, beta, Wk, Wq, Wv):
    input = np.asarray(input)
    B, S = input.shape
    voc, d = np.asarray(word_emb).shape
    assert d == D
    b_per_core = B // N_CORES

    apply_beta = bool(np.any(np.asarray(beta) != 0.0))
    in_maps = prepare_in_maps(input, word_emb, pos_emb, gamma, beta,
                              Wk, Wq, Wv, b_per_core)
    nc = _get_module(b_per_core, S, voc, apply_beta)

    res = run_bass_kernel_spmd(nc, in_maps, core_ids=list(range(N_CORES)))
    out = np.concatenate(
        [r["out"].reshape(b_per_core, S, D) for r in res.results], axis=0)
    return out
